# revision 1
# baseline (speedup 1.0000x reference)
"""DVAE encoder (batched DAG GRU message passing) on 8 trn2 NeuronCores.

Pure data-parallel over batch (256 graphs/core). Feature-major GRU compute
(features on partitions, batch on free dim) with weight-stationary bf16
matmuls at N=256. Key structure vs the naive version:
  - Bn (= w_ih_n @ one-hot x + b_ih_n) is a host-side gather, DMA'd per
    step; PE never computes it.
  - Diagonal adjacency mask tiles are host-built and DMA'd (per-step
    tiles), not generated on DVE.
  - gate/mapper matmuls run batch-major (hv chunks stationary, weights
    moving) so their PSUM output is batch-major: the message write is a
    single DVE multiply into msb and the old gm->msb PE transposes are
    gone. Gate bias and the vertex-id columns are folded into a per-step
    kc3 weight slice (const-1 row of hv).
  - Aggregation-prefix matmuls for step v+1 are emitted mid-step to fill
    the PE idle gap during the GRU pointwise chain.
  - Blend uses z' = sigmoid(-x): hv = z'*n + z*h, with z*h computed on
    the (otherwise idle) GpSimd engine off the critical chain.
  - r/z/n/g activations are single wide ACT ops; PE is pre-warmed with
    dummy matmuls during the initial weight DMA.
"""

import numpy as np

B, MAX_N, NVT, HS, NZ = 2048, 16, 8, 501, 56
NC_CORES = 8
BL = B // NC_CORES     # 256 per core
NBT = BL // 128        # 2 batch tiles

_CACHE = {}


def _tri(vn):
    return vn * (vn - 1) // 2


def _build_nc():
    import concourse.mybir as mybir
    import concourse.tile as tile
    from concourse import bacc

    F32 = mybir.dt.float32
    BF = mybir.dt.bfloat16
    AF = mybir.ActivationFunctionType

    nc = bacc.Bacc("TRN2", target_bir_lowering=False, debug=False,
                   num_devices=NC_CORES)

    d_wa = nc.dram_tensor("wa", [128, 4 * 1024], BF, kind="ExternalInput").ap()
    d_wc = nc.dram_tensor("wc", [128, 4 * 512], BF, kind="ExternalInput").ap()
    d_wgm = nc.dram_tensor("wgm", [128, 3 * 1024], BF, kind="ExternalInput").ap()
    d_wgm3 = nc.dram_tensor("wgm3", [128, 16 * 1024], BF, kind="ExternalInput").ap()
    d_bnf = nc.dram_tensor("bnf", [128, 16 * 1024], BF, kind="ExternalInput").ap()
    d_dmf = [nc.dram_tensor(f"dmf{bt}", [128, 120 * 128], BF,
                            kind="ExternalInput").ap() for bt in range(NBT)]
    d_xh = nc.dram_tensor("xh", [128, 16 * NBT * 9], BF, kind="ExternalInput").ap()
    d_wf = nc.dram_tensor("wf", [128, 4 * 112], BF, kind="ExternalInput").ap()
    d_fcb = nc.dram_tensor("fcb", [128, 1], F32, kind="ExternalInput").ap()
    d_id = nc.dram_tensor("ident", [128, 128], BF, kind="ExternalInput").ap()
    d_y = nc.dram_tensor("y", [112, BL], F32, kind="ExternalOutput").ap()

    with tile.TileContext(nc) as tc:
        with tc.tile_pool(name="wts", bufs=1) as wts, \
             tc.tile_pool(name="dbuf", bufs=2) as dbuf, \
             tc.tile_pool(name="state", bufs=1) as state, \
             tc.tile_pool(name="work", bufs=2) as work, \
             tc.tile_pool(name="hbmp", bufs=2) as hbmp, \
             tc.tile_pool(name="psA", bufs=2, space="PSUM") as psA, \
             tc.tile_pool(name="pagg", bufs=2, space="PSUM") as paggp, \
             tc.tile_pool(name="psT", bufs=2, space="PSUM") as psT:

            ident = wts.tile([128, 128], BF, tag="ident", name="ident")
            nc.sync.dma_start(out=ident[:], in_=d_id[:])

            # PE warmup during the weight DMA: keeps the HAM clock gate
            # open so step-0 matmuls run at full rate.
            pwarm = psA.tile([128, 4, 256], F32, tag="psA", name="pwarm")
            for i in range(36):
                nc.tensor.matmul(pwarm[:, 0, 0:128], ident[:], ident[:],
                                 start=True, stop=True)

            xh = wts.tile([128, 16 * NBT * 9], BF, tag="xh", name="xh")
            wa = wts.tile([128, 4 * 1024], BF, tag="wa", name="wa")
            wc = wts.tile([128, 4 * 512], BF, tag="wc", name="wc")
            wgm = wts.tile([128, 3 * 1024], BF, tag="wgm", name="wgm")
            wf = wts.tile([128, 4 * 112], BF, tag="wf", name="wf")
            fcb = wts.tile([128, 1], F32, tag="fcb", name="fcb")
            nc.sync.dma_start(out=xh[:], in_=d_xh[:])
            nc.sync.dma_start(out=wa[:], in_=d_wa[:])
            nc.sync.dma_start(out=wc[:], in_=d_wc[:])

            # double-buffered per-step tensors ([128, 4, 256] view of a
            # flat [128, 1024] dram slice)
            def fetch(tag, dram, v):
                t = dbuf.tile([128, 4, 256], BF, tag=tag, name=f"{tag}{v}")
                nc.sync.dma_start(out=t[:], in_=dram[:, v * 1024:(v + 1) * 1024])
                return t

            bn_cur = fetch("bn", d_bnf, 0)
            wgm3_cur = fetch("wgm3", d_wgm3, 0)
            for kc in range(3):
                nc.sync.dma_start(out=wgm[:, kc * 1024:(kc + 1) * 1024],
                                  in_=d_wgm[:, kc * 1024:(kc + 1) * 1024])

            # per-step mask tiles (vn = v+1 = 1..15), granular DMA deps;
            # small early-step masks go before the big weight tensors
            dmt = [[None] * MAX_N for _ in range(NBT)]

            def dm_fetch(vn):
                for bt in range(NBT):
                    t = wts.tile([128, vn * 128], BF, tag=f"dm{bt}_{vn}",
                                 name=f"dm{bt}_{vn}")
                    dmt[bt][vn] = t
                    nc.sync.dma_start(
                        out=t[:],
                        in_=d_dmf[bt][:, _tri(vn) * 128:(_tri(vn) + vn) * 128])

            for vn in range(1, 5):
                dm_fetch(vn)
            nc.sync.dma_start(out=wf[:], in_=d_wf[:])
            nc.sync.dma_start(out=fcb[:], in_=d_fcb[:])
            for vn in range(5, MAX_N):
                dm_fetch(vn)

            # messages, batch-major: [128b, u(16) * bt(2) * 512f]
            msb = state.tile([128, MAX_N * NBT * 512], BF, tag="msb",
                             name="msb")

            # initial hT: zeros + x rows, via batch-major hbm + transpose
            hT = work.tile([128, 4, 256], BF, tag="hT", name="hT0")
            for bt in range(NBT):
                hb = hbmp.tile([128, 512], BF, tag=f"hbm{bt}",
                               name=f"hbm{bt}_0")
                nc.vector.memset(hb[:], 0.0)
                nc.vector.tensor_copy(hb[:, 501:510], xh[:, bt * 9:bt * 9 + 9])
                ptp = psT.tile([128, 4, 128], BF, tag="ptp", name="ptp0")
                for kc in range(4):
                    nc.tensor.transpose(
                        ptp[:, kc, :], hb[:, kc * 128:kc * 128 + 128],
                        ident[:])
                nc.vector.tensor_copy(
                    hT[:, :, bt * 128:bt * 128 + 128], ptp[:])

            for v in range(MAX_N):
                vn = v + 1
                if v < MAX_N - 1:
                    # prefetch next-step tensors with maximum DMA lead
                    bn_nxt = fetch("bn", d_bnf, v + 1)
                    wgm3_nxt = fetch("wgm3", d_wgm3, v + 1)
                # ---- r, z, Cn matmuls (r first: longest pointwise path) ----
                ps_r = psA.tile([128, 4, 256], F32, tag="psA", name="ps_r")
                for mt in range(4):
                    for kc in range(4):
                        nc.tensor.matmul(
                            ps_r[:, mt, :],
                            wa[:, kc * 1024 + mt * 128:kc * 1024 + mt * 128 + 128],
                            hT[:, kc, :], start=(kc == 0), stop=(kc == 3))
                ps_z = psA.tile([128, 4, 256], F32, tag="psA", name="ps_z")
                for mt in range(4):
                    for kc in range(4):
                        nc.tensor.matmul(
                            ps_z[:, mt, :],
                            wa[:, kc * 1024 + 512 + mt * 128:
                               kc * 1024 + 512 + mt * 128 + 128],
                            hT[:, kc, :], start=(kc == 0), stop=(kc == 3))
                ps_c = psA.tile([128, 4, 256], F32, tag="psA", name="ps_c")
                for mt in range(4):
                    for kc in range(4):
                        nc.tensor.matmul(
                            ps_c[:, mt, :],
                            wc[:, kc * 512 + mt * 128:kc * 512 + mt * 128 + 128],
                            hT[:, kc, :], start=(kc == 0), stop=(kc == 3))

                pags = []
                if vn < MAX_N:
                    for bt in range(NBT):
                        pag = paggp.tile([128, 512], F32, tag="pagg",
                                         name="pag")
                        pags.append(pag)
                        # first half of the aggregation prefix fills the
                        # mid-step pointwise gap
                        for u in range(v // 2):
                            nc.tensor.matmul(
                                pag[:],
                                dmt[bt][vn][:, u * 128:u * 128 + 128],
                                msb[:, (u * NBT + bt) * 512:
                                    (u * NBT + bt) * 512 + 512],
                                start=(u == 0), stop=False)

                pdum = psT.tile([128, 4, 128], BF, tag="ptp",
                                name="pdum")
                for i in range(max(14, 60 - 8 * v)):
                    nc.tensor.transpose(pdum[:, 0, :], ident[:], ident[:])

                # ---- GRU pointwise, split per batch half to pipeline ----
                r = work.tile([128, 4, 256], BF, tag="r", name="r")
                z = work.tile([128, 4, 256], BF, tag="z", name="z")
                z2 = work.tile([128, 4, 256], BF, tag="z2", name="z2")
                u_t = work.tile([128, 4, 256], BF, tag="u_t", name="u_t")
                t_t = work.tile([128, 4, 256], BF, tag="t_t", name="t_t")
                n_t = work.tile([128, 4, 256], BF, tag="n_t", name="n_t")
                q_t = work.tile([128, 4, 256], BF, tag="q_t", name="q_t")
                p_t = work.tile([128, 4, 256], BF, tag="p_t", name="p_t")
                hv = work.tile([128, 4, 256], BF, tag="hv", name="hv")

                def bs(t, bt):
                    return t[:, :, bt * 128:(bt + 1) * 128]

                for bt in range(NBT):
                    nc.scalar.activation(bs(r, bt), bs(ps_r, bt), AF.Sigmoid)
                    nc.scalar.activation(bs(z, bt), bs(ps_z, bt), AF.Sigmoid)
                    nc.vector.tensor_mul(bs(u_t, bt), bs(r, bt), bs(ps_c, bt))
                    nc.vector.tensor_add(bs(t_t, bt), bs(u_t, bt),
                                         bs(bn_cur, bt))
                    nc.scalar.activation(bs(n_t, bt), bs(t_t, bt), AF.Tanh)
                    nc.gpsimd.tensor_mul(bs(q_t, bt), bs(z, bt), bs(hT, bt))
                    nc.vector.tensor_scalar(
                        bs(z2, bt), bs(z, bt), -1.0, 1.0,
                        op0=mybir.AluOpType.mult, op1=mybir.AluOpType.add)
                    nc.vector.tensor_mul(bs(p_t, bt), bs(z2, bt), bs(n_t, bt))
                    nc.vector.tensor_add(bs(hv, bt), bs(p_t, bt), bs(q_t, bt))

                if v < MAX_N - 1:

                    hT_next = work.tile([128, 4, 256], BF, tag="hT",
                                        name="hT")
                    hbm = []
                    for bt in range(NBT):
                        # ---- gate/mapper matmuls, batch-major ----
                        gmp = psA.tile([128, 4, 256], F32, tag="psA",
                                       name="gmp")
                        def gm_half(half):
                            for kc in range(3):
                                nc.tensor.matmul(
                                    gmp[:, half * 2:half * 2 + 2, :],
                                    hv[:, kc, bt * 128:bt * 128 + 128],
                                    wgm[:, kc * 1024 + half * 512:
                                        kc * 1024 + half * 512 + 512],
                                    start=(kc == 0), stop=False)
                            nc.tensor.matmul(
                                gmp[:, half * 2:half * 2 + 2, :],
                                hv[:, 3, bt * 128:bt * 128 + 128],
                                wgm3_cur[:, half * 2:half * 2 + 2, :],
                                start=False, stop=True)

                        gm_half(0)
                        g = work.tile([128, 512], BF, tag=f"g{bt}",
                                      name=f"g{bt}")
                        nc.scalar.activation(g[:], gmp[:, 0:2, :], AF.Sigmoid)
                        gm_half(1)
                        # second half of the aggregation prefix fills the
                        # step tail
                        for u in range(v // 2, v):
                            nc.tensor.matmul(
                                pags[bt][:],
                                dmt[bt][vn][:, u * 128:u * 128 + 128],
                                msb[:, (u * NBT + bt) * 512:
                                    (u * NBT + bt) * 512 + 512],
                                start=(u == 0), stop=False)
                        moff = (v * NBT + bt) * 512
                        nc.vector.tensor_mul(msb[:, moff:moff + 512],
                                             gmp[:, 2:4, :], g[:])
                        # ---- final aggregation term (this step's message) ----
                        nc.tensor.matmul(
                            pags[bt][:],
                            dmt[bt][vn][:, v * 128:v * 128 + 128],
                            msb[:, moff:moff + 512],
                            start=(v == 0), stop=True)
                        hb = hbmp.tile([128, 512], BF, tag=f"hbm{bt}",
                                       name=f"hbm{bt}")
                        hbm.append(hb)
                        ptp = psT.tile([128, 4, 128], BF, tag="ptp",
                                       name="ptp")
                        nc.scalar.copy(hb[:, 0:256], pags[bt][:, 0:256])
                        for kc in range(2):
                            nc.tensor.transpose(
                                ptp[:, kc, :],
                                hb[:, kc * 128:kc * 128 + 128], ident[:])
                        nc.scalar.copy(hb[:, 256:512], pags[bt][:, 256:512])
                        nc.vector.tensor_copy(
                            hb[:, 501:510],
                            xh[:, (vn * NBT + bt) * 9:(vn * NBT + bt) * 9 + 9])
                        for kc in range(2, 4):
                            nc.tensor.transpose(
                                ptp[:, kc, :],
                                hb[:, kc * 128:kc * 128 + 128], ident[:])
                        nc.vector.tensor_copy(
                            hT_next[:, :, bt * 128:bt * 128 + 128], ptp[:])

                    hT = hT_next
                    bn_cur, wgm3_cur = bn_nxt, wgm3_nxt
                else:
                    # ---- final FC ----
                    pf = psA.tile([128, 4, 256], F32, tag="psA", name="pf")
                    for kc in range(4):
                        nc.tensor.matmul(
                            pf[:112, 0, :], wf[:, kc * 112:kc * 112 + 112],
                            hv[:, kc, :], start=(kc == 0), stop=(kc == 3))
                    out_sb = work.tile([128, 256], F32, tag="out_sb",
                                       name="out_sb")
                    nc.scalar.activation(out_sb[:112, :], pf[:112, 0, :],
                                         AF.Identity, bias=fcb[:112, :])
                    nc.sync.dma_start(out=d_y[:], in_=out_sb[:112, :])

    nc.compile()
    return nc


def _prep_static(w_ih, w_hh, b_ih, b_hh, gate_w, gate_b, map_w,
                 fc1_w, fc1_b, fc2_w, fc2_b):
    import ml_dtypes
    f32 = np.float32
    bf16 = ml_dtypes.bfloat16
    bias = (b_ih + b_hh).astype(f32)

    WA = np.zeros((512, 1024), f32)
    WA[0:501, 0:501] = w_hh[0:501].T
    WA[501:509, 0:501] = w_ih[0:501].T
    WA[509, 0:501] = bias[0:501]
    WA[0:501, 512:1013] = w_hh[501:1002].T
    WA[501:509, 512:1013] = w_ih[501:1002].T
    WA[509, 512:1013] = bias[501:1002]
    # z-padding output 509: sigmoid(30) == 1 exactly in bf16, so
    # hv[509] = z*hT = const-1 propagates for the gate-bias fold
    WA[509, 1021] = 30.0
    WC = np.zeros((512, 512), f32)
    WC[0:501, 0:501] = w_hh[1002:1503].T
    WC[509, 0:501] = b_hh[1002:1503]
    WF = np.zeros((512, 112), f32)
    WF[0:501, 0:56] = fc1_w.T
    WF[0:501, 56:112] = fc2_w.T

    def ktile_flat(W, cols):
        return np.ascontiguousarray(
            W.reshape(4, 128, cols).transpose(1, 0, 2).reshape(128, 4 * cols)
        ).astype(bf16)

    # g/m moving weights: W^T layouts [K(features), N(outputs)]
    WGT = np.zeros((512, 512), f32)
    WGT[0:501, 0:501] = gate_w[:, 0:501].T
    WMT = np.zeros((512, 512), f32)
    WMT[0:501, 0:501] = map_w[:, 0:501].T
    wgm = np.zeros((128, 3 * 1024), f32)
    for kc in range(3):
        wgm[:, kc * 1024:kc * 1024 + 512] = WGT[kc * 128:(kc + 1) * 128]
        wgm[:, kc * 1024 + 512:(kc + 1) * 1024] = WMT[kc * 128:(kc + 1) * 128]
    wgm3 = np.zeros((128, 16 * 1024), f32)
    for v in range(16):
        blk = wgm3[:, v * 1024:(v + 1) * 1024]
        blk[0:117, 0:512] = WGT[384:501]
        blk[0:117, 512:1024] = WMT[384:501]
        blk[125, 0:501] = gate_b + gate_w[:, HS + v]
        blk[125, 512:512 + 501] = map_w[:, HS + v]

    fcb = np.zeros((128, 1), f32)
    fcb[0:56, 0] = fc1_b
    fcb[56:112, 0] = fc2_b
    ident = np.eye(128, dtype=f32).astype(bf16)
    return dict(wa=ktile_flat(WA, 1024), wc=ktile_flat(WC, 512),
                wgm=wgm.astype(bf16), wgm3=wgm3.astype(bf16),
                wf=ktile_flat(WF, 112), fcb=fcb, ident=ident)


def _prep_core(node_types, adj, w_ih, b_ih, core):
    import ml_dtypes
    f32 = np.float32
    bf16 = ml_dtypes.bfloat16
    off = core * BL
    nt = np.asarray(node_types[off:off + BL])       # [256, 16] int32
    ad = np.asarray(adj[off:off + BL], dtype=f32)   # [256, 16, 16]

    # x (one-hot + const-1), batch-major [128, 16*2*9]
    xh = np.zeros((128, 16 * NBT * 9), f32)
    for v in range(16):
        for bt in range(NBT):
            nb = nt[bt * 128:(bt + 1) * 128, v]
            base = (v * NBT + bt) * 9
            xh[:, base:base + 8] = (
                nb[:, None] == np.arange(NVT)[None, :]).astype(f32)
            xh[:, base + 8] = 1.0

    # Bn = w_ih_n @ x + b_ih_n, feature-major ktiles [128, 16*1024]
    W3 = np.asarray(w_ih[1002:1503], f32)           # [501, 8]
    B3 = np.asarray(b_ih[1002:1503], f32)
    bnf = np.zeros((128, 16 * 1024), f32)
    for v in range(16):
        BN = np.zeros((512, 256), f32)
        BN[0:501] = W3[:, nt[:, v]] + B3[:, None]
        bnf[:, v * 1024:(v + 1) * 1024] = (
            BN.reshape(4, 128, 256).transpose(1, 0, 2).reshape(128, 1024))

    # diagonal mask tiles per bt: [128, 120*128]
    dm = []
    ar = np.arange(128)
    for bt in range(NBT):
        blocks = np.zeros((128, 120, 128), f32)
        for vn in range(1, 16):
            for u in range(vn):
                blocks[ar, _tri(vn) + u, ar] = ad[bt * 128 + ar, vn, u]
        dm.append(np.ascontiguousarray(
            blocks.reshape(128, 120 * 128)).astype(bf16))

    return dict(xh=xh.astype(bf16), bnf=bnf.astype(bf16),
                dmf0=dm[0], dmf1=dm[1])


def _prep_all(inputs):
    static = _prep_static(
        np.asarray(inputs["w_ih"], np.float32),
        np.asarray(inputs["w_hh"], np.float32),
        np.asarray(inputs["b_ih"], np.float32),
        np.asarray(inputs["b_hh"], np.float32),
        np.asarray(inputs["gate_w"], np.float32),
        np.asarray(inputs["gate_b"], np.float32),
        np.asarray(inputs["map_w"], np.float32),
        np.asarray(inputs["fc1_w"], np.float32),
        np.asarray(inputs["fc1_b"], np.float32),
        np.asarray(inputs["fc2_w"], np.float32),
        np.asarray(inputs["fc2_b"], np.float32))
    in_maps = []
    for c in range(NC_CORES):
        m = dict(static)
        m.update(_prep_core(inputs["node_types"], inputs["adj"],
                            np.asarray(inputs["w_ih"], np.float32),
                            np.asarray(inputs["b_ih"], np.float32), c))
        in_maps.append(m)
    return in_maps


def kernel(node_types, adj, w_ih, w_hh, b_ih, b_hh, gate_w, gate_b, map_w,
           fc1_w, fc1_b, fc2_w, fc2_b):
    from concourse.bass_utils import run_bass_kernel_spmd

    if "nc" not in _CACHE:
        _CACHE["nc"] = _build_nc()
    nc = _CACHE["nc"]

    in_maps = _prep_all(dict(
        node_types=node_types, adj=adj, w_ih=w_ih, w_hh=w_hh, b_ih=b_ih,
        b_hh=b_hh, gate_w=gate_w, gate_b=gate_b, map_w=map_w,
        fc1_w=fc1_w, fc1_b=fc1_b, fc2_w=fc2_w, fc2_b=fc2_b))

    res = run_bass_kernel_spmd(nc, in_maps, core_ids=list(range(NC_CORES)))
    ys = [res.results[c]["y"] for c in range(NC_CORES)]   # each [112, 256]
    out = np.concatenate(ys, axis=1).T                     # [2048, 112]
    return np.ascontiguousarray(out.astype(np.float32))

